# revision 4
# baseline (speedup 1.0000x reference)
"""D2Q9 lattice-Boltzmann solver step (collision + moments + streaming) on 8
Trainium2 NeuronCores.

Sharding: the (Y, X) grid is split along Y into 8 contiguous slabs of 256
rows, one per core. All moment/collision math is local per cell; the
periodic-shift streaming step is realized purely through output DMA
addressing (write F_post row y to output row y-EY, column x+EX mod X). The
six F_str rows per core that fall outside the core's own output slab
(EY=+1 planes at the top edge, EY=-1 planes at the bottom edge) are written
to a small per-core `extra` tensor and placed by the host gather, so no
input halo or device-to-device communication is needed at all.

Per core the program runs 2 row-supertiles x 3 x-blocks. Per block:
  - load F/Feq planes as (128, xb) tiles; load G in a q-on-partition layout
    (9q x 14row groups) and reduce Esum = sum_q G on the TensorEngine with
    0/1 fp32 weights accumulated into PSUM,
  - d = F - Feq, r = |d| / (Feq + 1e-10) (fused abs+divide), EPS
    accumulated in ascending q order (threshold-critical vs reference),
  - rho / ux / uy via shared-subexpression adds, then per-cell fields
    (T, w, tau, omega, omegaT, qx, qy ...) split across DVE / GpSimd / ACT,
  - F_post = F - omega * d, streamed out with shifted DMA addressing.
"""
from contextlib import ExitStack

import numpy as np

# ---------------- problem constants (hardcoded per contract) ----------------
Qn, Y, X = 9, 2048, 2048
N_CORES = 8
RPC = Y // N_CORES  # 256 interior rows per core
XBLOCKS = (768, 768, 512)  # x-block widths (sum = X)
EX = [1, 0, -1, 0, 1, -1, -1, 1, 0]
EY = [0, 1, 0, -1, 1, 1, -1, -1, 0]
# G-group layout for the Esum matmuls: (row offset, nrows); 9*14+9*2 = 128 rows
GROUPS = [(14 * g, 14) for g in range(9)] + [(126, 2)]
EXTRA_TOP = {1: 0, 4: 1, 5: 2}  # EY=+1: F_str global row y0-1  -> extra[idx]
EXTRA_BOT = {3: 3, 6: 4, 7: 5}  # EY=-1: F_str global row y0+256 -> extra[idx]

# ---- constants replicated in f32 exactly as the jax reference computes ----
_F = np.float32
ICV32 = float(_F(1.4 - 1.0))               # 0.40000000596... (f32 of 0.4-ish)
C_T = ICV32 / 2.0                          # T = C_T * (E2 - uu); 2*C_T == ICV32
K1 = float(_F(_F(1.35) * _F(0.01)))        # m2 = K1 * r1 + K0  (= 1.35*tau_DL - 1)
K0 = float(_F(_F(1.35) * _F(0.5)) - _F(1.0))
C1T = float(_F(1.0) / _F(0.71))            # tauT = C1T * tmw + C0T
C0T = float(_F(0.5) + _F(_F(0.5) * _F(1.0) / _F(0.71)))
EPS_BIAS = float(_F(1e-10))

_CACHE = {}


def _esum_weights():
    """lhsT weights (10, 126, 128) f32: W[g][(q*rows+dy), 14*g+dy] = 1."""
    W = np.zeros((10, 126, 128), np.float32)
    for g, (r0, rows) in enumerate(GROUPS):
        for q in range(Qn):
            for dy in range(rows):
                W[g, q * rows + dy, r0 + dy] = 1.0
    return W


def build_program():
    import concourse.bass as bass  # noqa: F401  (AP types via aps)
    import concourse.tile as tile
    from concourse import bacc, mybir

    f32 = mybir.dt.float32
    OP = mybir.AluOpType
    AF = mybir.ActivationFunctionType

    nc = bacc.Bacc("TRN2", target_bir_lowering=False, debug=False,
                   enable_asserts=False, num_devices=N_CORES)
    # extra const AP used as ACT bias (e = Feq + 1e-10)
    _ct = nc.alloc_sbuf_tensor("const-eps10", [128, 1], f32)
    nc.gpsimd.memset(_ct.ap(), EPS_BIAS)
    nc.const_aps.aps[(f32, EPS_BIAS)] = _ct.ap()
    nc.all_engine_barrier()
    F_ap = nc.dram_tensor("F", [Qn, RPC, X], f32, kind="ExternalInput").ap()
    G_ap = nc.dram_tensor("G", [Qn, RPC, X], f32, kind="ExternalInput").ap()
    Feq_ap = nc.dram_tensor("Feq", [Qn, RPC, X], f32, kind="ExternalInput").ap()
    W_ap = nc.dram_tensor("W", [10, 126, 128], f32, kind="ExternalInput").ap()
    out_ap = nc.dram_tensor("out", [26, RPC, X], f32, kind="ExternalOutput").ap()
    ext_ap = nc.dram_tensor("extra", [6, X], f32, kind="ExternalOutput").ap()

    with tile.TileContext(nc) as tc, ExitStack() as ctx:
        pW = ctx.enter_context(tc.tile_pool(name="w", bufs=1))
        pF = ctx.enter_context(tc.tile_pool(name="pf", bufs=1))
        pD = ctx.enter_context(tc.tile_pool(name="pd", bufs=1))
        pL = ctx.enter_context(tc.tile_pool(name="pl", bufs=3))   # feq / G rotating
        pT = ctx.enter_context(tc.tile_pool(name="pt", bufs=2))   # e / r rotating
        pC = ctx.enter_context(tc.tile_pool(name="pc", bufs=1))   # per-cell tags
        pP = ctx.enter_context(tc.tile_pool(name="pp", bufs=2, space="PSUM"))

        # stationary Esum weights, loaded once
        Wt = []
        for g, (_, rows) in enumerate(GROUPS):
            parts = Qn * rows
            wt = pW.tile([parts, 128], f32, tag=f"W{g}")
            nc.sync.dma_start(wt[:], W_ap[g, :parts, :])
            Wt.append(wt)

        def block(r0, x0, xb):
            # ---------------- loads + Esum matmul ----------------
            Ft = []
            for q in range(Qn):
                fq = pF.tile([128, xb], f32, tag=f"F{q}")
                nc.sync.dma_start(fq[:], F_ap[q, r0:r0 + 128, x0:x0 + xb])
                Ft.append(fq)

            es = pP.tile([128, xb], f32, tag="esum")
            for g, (gr0, rows) in enumerate(GROUPS):
                parts = Qn * rows
                gt = pL.tile([parts, xb], f32, tag="g")
                nc.sync.dma_start(gt[:], G_ap[:, r0 + gr0:r0 + gr0 + rows, x0:x0 + xb])
                for n0 in range(0, xb, 512):
                    n1 = min(xb, n0 + 512)
                    nc.tensor.matmul(es[:, n0:n1], Wt[g][:parts, :], gt[:, n0:n1],
                                     start=(g == 0), stop=(g == len(GROUPS) - 1))

            # ---------------- per-q: d, e, r, EPS acc ----------------
            # r = |F-Feq| * recip(Feq + 1e-10); vector.reciprocal is
            # HW-verified bit-exact IEEE 1/x, so r is within 1 ulp of the
            # reference's true divide. Accumulate in ascending q order to
            # match the reference reduction order (threshold-critical).
            acc = pC.tile([128, xb], f32, tag="acc")
            Dt = []
            for q in range(Qn):
                feq = pL.tile([128, xb], f32, tag="feq")
                nc.sync.dma_start(feq[:], Feq_ap[q, r0:r0 + 128, x0:x0 + xb])
                d = pD.tile([128, xb], f32, tag=f"d{q}")
                nc.vector.tensor_tensor(d[:], Ft[q][:], feq[:], OP.subtract)
                Dt.append(d)
                e = pT.tile([128, xb], f32, tag="e")
                nc.scalar.activation(e[:], feq[:], AF.Identity, bias=EPS_BIAS)
                nc.vector.reciprocal(e[:], e[:])
                ad = pT.tile([128, xb], f32, tag="ad")
                nc.scalar.activation(ad[:], d[:], AF.Abs)
                if q == 0:
                    nc.vector.tensor_tensor(acc[:], ad[:], e[:], OP.mult)
                else:
                    nc.vector.tensor_tensor(ad[:], ad[:], e[:], OP.mult)
                    nc.vector.tensor_tensor(acc[:], acc[:], ad[:], OP.add)

            # ---------------- rho / ux / uy ----------------
            sxp = pC.tile([128, xb], f32, tag="sxp")   # F0+F4+F7
            nc.gpsimd.tensor_tensor(sxp[:], Ft[0][:], Ft[4][:], OP.add)
            nc.gpsimd.tensor_tensor(sxp[:], sxp[:], Ft[7][:], OP.add)
            sxm = pC.tile([128, xb], f32, tag="sxm")   # F2+F5+F6
            nc.gpsimd.tensor_tensor(sxm[:], Ft[2][:], Ft[5][:], OP.add)
            nc.gpsimd.tensor_tensor(sxm[:], sxm[:], Ft[6][:], OP.add)
            s138 = pC.tile([128, xb], f32, tag="s138")  # F1+F3+F8
            nc.gpsimd.tensor_tensor(s138[:], Ft[1][:], Ft[3][:], OP.add)
            nc.gpsimd.tensor_tensor(s138[:], s138[:], Ft[8][:], OP.add)
            rho = pC.tile([128, xb], f32, tag="rho")
            nc.gpsimd.tensor_tensor(rho[:], sxp[:], sxm[:], OP.add)
            nc.gpsimd.tensor_tensor(rho[:], rho[:], s138[:], OP.add)
            uxn = pC.tile([128, xb], f32, tag="uxn")
            nc.vector.tensor_tensor(uxn[:], sxp[:], sxm[:], OP.subtract)
            syp = pC.tile([128, xb], f32, tag="syp")   # F1+F4+F5
            nc.gpsimd.tensor_tensor(syp[:], Ft[4][:], Ft[5][:], OP.add)
            nc.gpsimd.tensor_tensor(syp[:], syp[:], Ft[1][:], OP.add)
            sym = pC.tile([128, xb], f32, tag="sym")   # F3+F6+F7
            nc.gpsimd.tensor_tensor(sym[:], Ft[6][:], Ft[7][:], OP.add)
            nc.gpsimd.tensor_tensor(sym[:], sym[:], Ft[3][:], OP.add)
            uyn = pC.tile([128, xb], f32, tag="uyn")
            nc.vector.tensor_tensor(uyn[:], syp[:], sym[:], OP.subtract)

            # ---------------- per-cell fields ----------------
            invr = pC.tile([128, xb], f32, tag="invr")
            nc.vector.reciprocal(invr[:], rho[:])
            ux = pC.tile([128, xb], f32, tag="ux")
            nc.vector.tensor_tensor(ux[:], uxn[:], invr[:], OP.mult)
            uy = pC.tile([128, xb], f32, tag="uy")
            nc.vector.tensor_tensor(uy[:], uyn[:], invr[:], OP.mult)
            E2 = pC.tile([128, xb], f32, tag="E2")     # = 2*E
            nc.vector.tensor_tensor(E2[:], es[:], invr[:], OP.mult)
            sqx = pC.tile([128, xb], f32, tag="sqx")
            nc.scalar.activation(sqx[:], ux[:], AF.Square)
            sqy = pC.tile([128, xb], f32, tag="sqy")
            nc.scalar.activation(sqy[:], uy[:], AF.Square)
            nc.gpsimd.tensor_tensor(sqx[:], sqx[:], sqy[:], OP.add)      # uu
            nc.vector.tensor_tensor(sqx[:], E2[:], sqx[:], OP.subtract)  # E2-uu
            T = pC.tile([128, xb], f32, tag="T")
            nc.vector.tensor_scalar(T[:], sqx[:], C_T, 1e-6, OP.mult, OP.max)
            omT = pC.tile([128, xb], f32, tag="omT")   # 1 - T
            nc.scalar.activation(omT[:], T[:], AF.Copy, bias=1.0, scale=-1.0)
            sqT = pC.tile([128, xb], f32, tag="sqT")
            nc.scalar.activation(sqT[:], T[:], AF.Square)
            wa = pC.tile([128, xb], f32, tag="wa")     # 0.5*T*(1-T)
            nc.vector.scalar_tensor_tensor(wa[:], T[:], 0.5, omT[:], OP.mult, OP.mult)
            nc.scalar.mul(sqT[:], sqT[:], 0.25)        # wb = 0.25*T^2
            wc = pC.tile([128, xb], f32, tag="wc")     # (1-T)^2
            nc.scalar.activation(wc[:], omT[:], AF.Square)
            h = pC.tile([128, xb], f32, tag="h")       # E2 + 2T  (= 2*(E+T))
            nc.vector.scalar_tensor_tensor(h[:], T[:], 2.0, E2[:], OP.mult, OP.add)
            nc.gpsimd.tensor_tensor(h[:], rho[:], h[:], OP.mult)         # rhoH2
            qx = pC.tile([128, xb], f32, tag="qx")
            nc.gpsimd.tensor_tensor(qx[:], h[:], ux[:], OP.mult)
            qy = pC.tile([128, xb], f32, tag="qy")
            nc.gpsimd.tensor_tensor(qy[:], h[:], uy[:], OP.mult)
            # tau / omega / omegaT
            nc.gpsimd.tensor_tensor(invr[:], rho[:], T[:], OP.mult)      # rho*T
            r1 = pC.tile([128, xb], f32, tag="r1")
            nc.vector.reciprocal(r1[:], invr[:])
            nc.vector.tensor_scalar(r1[:], r1[:], K1, K0, OP.mult, OP.add)  # m2
            mask = pC.tile([128, xb], f32, tag="mask")
            nc.vector.tensor_scalar(mask[:], acc[:], 9.0, None, OP.is_lt)
            tmw = pC.tile([128, xb], f32, tag="tmw")   # (tau-1) = m2*mask
            nc.vector.tensor_tensor(tmw[:], r1[:], mask[:], OP.mult)
            tau = pC.tile([128, xb], f32, tag="tau")
            nc.scalar.activation(tau[:], tmw[:], AF.Identity, bias=1.0)
            omg = pC.tile([128, xb], f32, tag="omg")
            nc.vector.reciprocal(omg[:], tau[:])
            nc.vector.tensor_scalar(tmw[:], tmw[:], C1T, C0T, OP.mult, OP.add)  # tauT
            omgT = pC.tile([128, xb], f32, tag="omgT")
            nc.vector.reciprocal(omgT[:], tmw[:])
            nc.scalar.mul(E2[:], E2[:], 0.5)           # E output

            # ---------------- F_post + streaming output ----------------
            for q in range(Qn):
                nc.gpsimd.tensor_tensor(Dt[q][:], omg[:], Dt[q][:], OP.mult)
                nc.vector.tensor_tensor(Ft[q][:], Ft[q][:], Dt[q][:], OP.subtract)

            # column segments for the periodic x shift
            def csegs(t):
                if t == 0:
                    return [(0, xb, x0)]
                if t == 1:
                    if x0 + xb == X:
                        return [(0, xb - 1, x0 + 1), (xb - 1, 1, 0)]
                    return [(0, xb, x0 + 1)]
                if x0 == 0:
                    return [(0, 1, X - 1), (1, xb - 1, 0)]
                return [(0, xb, x0 - 1)]

            for q in range(Qn):
                s = EY[q]
                rsegs = []  # (part0, nparts, kind, dstrow)
                if s == 1 and r0 == 0:
                    rsegs = [(0, 1, "x", EXTRA_TOP[q]), (1, 127, "m", 0)]
                elif s == -1 and r0 == 128:
                    rsegs = [(0, 127, "m", r0 + 1), (127, 1, "x", EXTRA_BOT[q])]
                else:
                    rsegs = [(0, 128, "m", r0 - s)]
                for (p0, np_, kind, dr) in rsegs:
                    for (c0, w, dc) in csegs(EX[q]):
                        src = Ft[q][p0:p0 + np_, c0:c0 + w]
                        if kind == "m":
                            nc.sync.dma_start(out_ap[q, dr:dr + np_, dc:dc + w], src)
                        else:
                            nc.sync.dma_start(ext_ap[dr, dc:dc + w], src)

            # w channels (broadcast 4/4/1) + field channels
            for ch in (9, 10, 11, 12):
                nc.sync.dma_start(out_ap[ch, r0:r0 + 128, x0:x0 + xb], wa[:])
            for ch in (13, 14, 15, 16):
                nc.sync.dma_start(out_ap[ch, r0:r0 + 128, x0:x0 + xb], sqT[:])
            nc.sync.dma_start(out_ap[17, r0:r0 + 128, x0:x0 + xb], wc[:])
            for ch, t in ((18, rho), (19, ux), (20, uy), (21, E2), (22, T),
                          (23, qx), (24, qy), (25, omgT)):
                nc.sync.dma_start(out_ap[ch, r0:r0 + 128, x0:x0 + xb], t[:])

        for r0 in (0, 128):
            x0 = 0
            for xb in XBLOCKS:
                block(r0, x0, xb)
                x0 += xb

    nc.compile()
    return nc


def _get_program():
    if "nc" not in _CACHE:
        _CACHE["nc"] = build_program()
    return _CACHE["nc"]


def kernel(F, G, Feq):
    from concourse.bass_utils import run_bass_kernel_spmd

    F = np.ascontiguousarray(np.asarray(F, np.float32))
    G = np.ascontiguousarray(np.asarray(G, np.float32))
    Feq = np.ascontiguousarray(np.asarray(Feq, np.float32))
    nc = _get_program()
    W = _esum_weights()
    in_maps = []
    for c in range(N_CORES):
        sl = slice(c * RPC, (c + 1) * RPC)
        in_maps.append({"F": F[:, sl, :], "G": G[:, sl, :], "Feq": Feq[:, sl, :],
                        "W": W})
    res = run_bass_kernel_spmd(nc, in_maps, core_ids=list(range(N_CORES)))
    out = np.empty((26, Y, X), np.float32)
    for c in range(N_CORES):
        out[:, c * RPC:(c + 1) * RPC, :] = res.results[c]["out"]
    for c in range(N_CORES):
        ex = res.results[c]["extra"]
        for q, i in EXTRA_TOP.items():
            out[q, (c * RPC - 1) % Y, :] = ex[i]
        for q, i in EXTRA_BOT.items():
            out[q, ((c + 1) * RPC) % Y, :] = ex[i]
    return out


# revision 9
# speedup vs baseline: 1.0666x; 1.0666x over previous
"""D2Q9 lattice-Boltzmann solver step (collision + moments + streaming) on 8
Trainium2 NeuronCores.

Sharding: the (Y, X) grid is split along Y into 8 contiguous slabs of 256
rows, one per core. All moment/collision math is local per cell; the
periodic-shift streaming step is realized purely through output DMA
addressing (write F_post row y to output row y-EY, column x+EX mod X). The
six F_str rows per core that fall outside the core's own output slab
(EY=+1 planes at the top edge, EY=-1 planes at the bottom edge) are written
to a small per-core `extra` tensor and placed by the host gather, so no
input halo or device-to-device communication is needed at all.

Per core the program runs 2 row-supertiles x 3 x-blocks. Per block:
  - one merged DMA each for the 9 F planes and 9 Feq planes into per-q
    arenas; G loaded in a q-on-partition layout (9q x 14row groups, 2 DMAs)
    and reduced Esum = sum_q G on the TensorEngine with 0/1 fp32 weights
    accumulated into PSUM,
  - d = F - Feq, r = |d| * recip(Feq + 1e-10) with the bit-exact DVE
    reciprocal; EPS accumulated in ascending q order (threshold-critical
    vs the reference — measured margin is only ~2e-7 relative),
  - rho / ux / uy via shared-subexpression adds on GpSimd; smooth-field
    reciprocals (1/rho, tau path) on the ACT spline engine (~1e-5 rel err,
    none of them feed the EPS mask),
  - F_post = F - omega * d, streamed out with shifted DMA addressing;
    input DMAs issue from the SP queue, output DMAs from the ACT queue.
"""
from contextlib import ExitStack

import numpy as np

# ---------------- problem constants (hardcoded per contract) ----------------
Qn, Y, X = 9, 2048, 2048
N_CORES = 8
RPC = Y // N_CORES  # 256 interior rows per core
XBLOCKS = (512, 512, 512, 512)  # x-block widths (sum = X)
EX = [1, 0, -1, 0, 1, -1, -1, 1, 0]
EY = [0, 1, 0, -1, 1, 1, -1, -1, 0]
# G-group layout for the Esum matmuls: (row offset, nrows); 9*14+9*2 = 128 rows
GROUPS = [(14 * g, 14) for g in range(9)] + [(126, 2)]
EXTRA_TOP = {1: 0, 4: 1, 5: 2}  # EY=+1: F_str global row y0-1  -> extra[idx]
EXTRA_BOT = {3: 3, 6: 4, 7: 5}  # EY=-1: F_str global row y0+256 -> extra[idx]

# ---- constants replicated in f32 exactly as the jax reference computes ----
_F = np.float32
ICV32 = float(_F(1.4 - 1.0))               # 0.40000000596... (f32 of 0.4-ish)
C_T = ICV32 / 2.0                          # T = C_T * (E2 - uu); 2*C_T == ICV32
K1 = float(_F(_F(1.35) * _F(0.01)))        # tau-1 = (K1/(rho T) + K0) * mask
K0 = float(_F(_F(1.35) * _F(0.5)) - _F(1.0))
INV_K1 = float(_F(1.0) / _F(K1))
C1T = float(_F(1.0) / _F(0.71))            # tauT = C1T * tmw + C0T
C0T = float(_F(0.5) + _F(_F(0.5) * _F(1.0) / _F(0.71)))
EPS_BIAS = float(_F(1e-10))

_CACHE = {}


def _esum_weights():
    """lhsT weights (10, 126, 128) f32: W[g][(q*rows+dy), 14*g+dy] = 1."""
    W = np.zeros((10, 126, 128), np.float32)
    for g, (r0, rows) in enumerate(GROUPS):
        for q in range(Qn):
            for dy in range(rows):
                W[g, q * rows + dy, r0 + dy] = 1.0
    return W


def build_program():
    import concourse.bass as bass  # noqa: F401
    import concourse.tile as tile
    from concourse import bacc, mybir

    f32 = mybir.dt.float32
    OP = mybir.AluOpType
    AF = mybir.ActivationFunctionType

    nc = bacc.Bacc("TRN2", target_bir_lowering=False, debug=False,
                   enable_asserts=False, num_devices=N_CORES)
    # extra const AP used as ACT bias (e = Feq + 1e-10)
    _ct = nc.alloc_sbuf_tensor("const-eps10", [128, 1], f32)
    nc.gpsimd.memset(_ct.ap(), EPS_BIAS)
    nc.const_aps.aps[(f32, EPS_BIAS)] = _ct.ap()
    nc.all_engine_barrier()

    F_ap = nc.dram_tensor("F", [Qn, RPC, X], f32, kind="ExternalInput").ap()
    G_ap = nc.dram_tensor("G", [Qn, RPC, X], f32, kind="ExternalInput").ap()
    Feq_ap = nc.dram_tensor("Feq", [Qn, RPC, X], f32, kind="ExternalInput").ap()
    W_ap = nc.dram_tensor("W", [10, 126, 128], f32, kind="ExternalInput").ap()
    out_ap = nc.dram_tensor("out", [26, RPC, X], f32, kind="ExternalOutput").ap()
    ext_ap = nc.dram_tensor("extra", [6, X], f32, kind="ExternalOutput").ap()

    def act_recip(out, in_, bias=0.0, scale=1.0):
        """Raw ACT-engine reciprocal: out = 1/(scale*in + bias).

        Spline-table implementation, measured <=1.2e-5 relative error —
        used only for smooth fields that never feed the EPS threshold.
        """
        nc.scalar.add_instruction(mybir.InstActivation(
            name=nc.get_next_instruction_name(),
            func=AF.Reciprocal,
            ins=[nc.scalar.lower_ap(in_),
                 mybir.ImmediateValue(dtype=f32, value=float(bias)),
                 mybir.ImmediateValue(dtype=f32, value=float(scale)),
                 mybir.ImmediateValue(dtype=f32, value=0.0)],
            outs=[nc.scalar.lower_ap(out)],
        ))

    with tile.TileContext(nc) as tc, ExitStack() as ctx:
        pW = ctx.enter_context(tc.tile_pool(name="w", bufs=1))
        pF = ctx.enter_context(tc.tile_pool(name="pf", bufs=2))    # F arena
        pQ = ctx.enter_context(tc.tile_pool(name="pq", bufs=2))    # Feq arena
        pD = ctx.enter_context(tc.tile_pool(name="pd", bufs=1))    # d tiles
        pL = ctx.enter_context(tc.tile_pool(name="pl", bufs=3))    # G group tiles
        pT = ctx.enter_context(tc.tile_pool(name="pt", bufs=2))    # e / ad rotating
        pC = ctx.enter_context(tc.tile_pool(name="pc", bufs=1))    # per-cell tags
        pP = ctx.enter_context(tc.tile_pool(name="pp", bufs=2, space="PSUM"))

        # stationary Esum weights, loaded once
        Wt = []
        for g, (_, rows) in enumerate(GROUPS):
            parts = Qn * rows
            wt = pW.tile([parts, 128], f32, tag=f"W{g}")
            nc.sync.dma_start(wt[:], W_ap[g, :parts, :])
            Wt.append(wt)

        def block(r0, x0, xb):
            # ---------------- merged loads ----------------
            farena = pF.tile([128, Qn * xb], f32, tag="farena")
            nc.sync.dma_start(
                farena[:].rearrange("p (q x) -> p q x", q=Qn),
                F_ap[:, r0:r0 + 128, x0:x0 + xb].rearrange("q r x -> r q x"))
            Ft = [farena[:, q * xb:(q + 1) * xb] for q in range(Qn)]

            qarena = pQ.tile([128, Qn * xb], f32, tag="qarena")
            nc.sync.dma_start(
                qarena[:].rearrange("p (q x) -> p q x", q=Qn),
                Feq_ap[:, r0:r0 + 128, x0:x0 + xb].rearrange("q r x -> r q x"))
            Feqt = [qarena[:, q * xb:(q + 1) * xb] for q in range(Qn)]

            es = pP.tile([128, xb], f32, tag="esum")
            for g, (gr0, rows) in enumerate(GROUPS):
                parts = Qn * rows
                gt = pL.tile([parts, xb], f32, tag="g")
                nc.sync.dma_start(gt[:], G_ap[:, r0 + gr0:r0 + gr0 + rows, x0:x0 + xb])
                for n0 in range(0, xb, 512):
                    n1 = min(xb, n0 + 512)
                    nc.tensor.matmul(es[:, n0:n1], Wt[g][:parts, :], gt[:parts, n0:n1],
                                     start=(g == 0), stop=(g == 9))

            # ---------------- per-q: d, e=recip(Feq+1e-10), EPS acc ----------
            # r = |F-Feq| * recip(Feq + 1e-10); vector.reciprocal is
            # HW-verified bit-exact IEEE 1/x; accumulate in ascending q order
            # (threshold-critical: the actual margin min|EPS-1| is ~2e-7).
            acc = pC.tile([128, xb], f32, tag="acc")
            Dt = []
            for q in range(Qn):
                d = pD.tile([128, xb], f32, tag=f"d{q}")
                nc.vector.tensor_tensor(d[:], Ft[q][:], Feqt[q][:], OP.subtract)
                Dt.append(d)
                e = pT.tile([128, xb], f32, tag="e")
                nc.scalar.activation(e[:], Feqt[q][:], AF.Identity, bias=EPS_BIAS)
                nc.vector.reciprocal(e[:], e[:])
                ad = pT.tile([128, xb], f32, tag="ad")
                nc.scalar.activation(ad[:], d[:], AF.Abs)
                if q == 0:
                    nc.vector.tensor_tensor(acc[:], ad[:], e[:], OP.mult)
                else:
                    nc.vector.tensor_tensor(ad[:], ad[:], e[:], OP.mult)
                    nc.vector.tensor_tensor(acc[:], acc[:], ad[:], OP.add)

            # ---------------- rho / ux / uy (GpSimd) ----------------
            sxp = pC.tile([128, xb], f32, tag="tmpA")   # F0+F4+F7
            nc.gpsimd.tensor_tensor(sxp[:], Ft[0][:], Ft[4][:], OP.add)
            nc.gpsimd.tensor_tensor(sxp[:], sxp[:], Ft[7][:], OP.add)
            sxm = pC.tile([128, xb], f32, tag="tmpB")   # F2+F5+F6
            nc.gpsimd.tensor_tensor(sxm[:], Ft[2][:], Ft[5][:], OP.add)
            nc.gpsimd.tensor_tensor(sxm[:], sxm[:], Ft[6][:], OP.add)
            s138 = pC.tile([128, xb], f32, tag="tmpC")  # F1+F3+F8
            nc.gpsimd.tensor_tensor(s138[:], Ft[1][:], Ft[3][:], OP.add)
            nc.gpsimd.tensor_tensor(s138[:], s138[:], Ft[8][:], OP.add)
            rho = pC.tile([128, xb], f32, tag="rho")
            nc.gpsimd.tensor_tensor(rho[:], sxp[:], sxm[:], OP.add)
            nc.gpsimd.tensor_tensor(rho[:], rho[:], s138[:], OP.add)
            uxn = pC.tile([128, xb], f32, tag="uxn")
            nc.gpsimd.tensor_tensor(uxn[:], sxp[:], sxm[:], OP.subtract)
            syp = pC.tile([128, xb], f32, tag="tmpC")   # F1+F4+F5
            nc.gpsimd.tensor_tensor(syp[:], Ft[4][:], Ft[5][:], OP.add)
            nc.gpsimd.tensor_tensor(syp[:], syp[:], Ft[1][:], OP.add)
            sym = pC.tile([128, xb], f32, tag="tmpB")   # F3+F6+F7
            nc.gpsimd.tensor_tensor(sym[:], Ft[6][:], Ft[7][:], OP.add)
            nc.gpsimd.tensor_tensor(sym[:], sym[:], Ft[3][:], OP.add)
            uyn = pC.tile([128, xb], f32, tag="uyn")
            nc.gpsimd.tensor_tensor(uyn[:], syp[:], sym[:], OP.subtract)

            # ---------------- per-cell fields ----------------
            invr = pC.tile([128, xb], f32, tag="invr")
            act_recip(invr[:], rho[:])                 # ~1e-5, smooth-only
            ux = pC.tile([128, xb], f32, tag="ux")
            nc.gpsimd.tensor_tensor(ux[:], uxn[:], invr[:], OP.mult)
            uy = pC.tile([128, xb], f32, tag="uy")
            nc.gpsimd.tensor_tensor(uy[:], uyn[:], invr[:], OP.mult)
            E2 = pC.tile([128, xb], f32, tag="E2")     # = 2*E
            nc.vector.tensor_tensor(E2[:], es[:], invr[:], OP.mult)
            sqx = pC.tile([128, xb], f32, tag="sqx")
            nc.scalar.activation(sqx[:], ux[:], AF.Square)
            sqy = pC.tile([128, xb], f32, tag="sqy")
            nc.scalar.activation(sqy[:], uy[:], AF.Square)
            nc.gpsimd.tensor_tensor(sqx[:], sqx[:], sqy[:], OP.add)      # uu
            nc.vector.tensor_tensor(sqx[:], E2[:], sqx[:], OP.subtract)  # E2-uu
            T = pC.tile([128, xb], f32, tag="T")
            nc.vector.tensor_scalar(T[:], sqx[:], C_T, 1e-6, OP.mult, OP.max)
            omT = pC.tile([128, xb], f32, tag="omT")   # 1 - T
            nc.scalar.activation(omT[:], T[:], AF.Copy, bias=1.0, scale=-1.0)
            sqT = pC.tile([128, xb], f32, tag="sqT")
            nc.scalar.activation(sqT[:], T[:], AF.Square)
            wa = pC.tile([128, xb], f32, tag="wa")     # 0.5*T*(1-T)
            nc.vector.scalar_tensor_tensor(wa[:], T[:], 0.5, omT[:], OP.mult, OP.mult)
            nc.scalar.mul(sqT[:], sqT[:], 0.25)        # wb = 0.25*T^2
            wc = pC.tile([128, xb], f32, tag="wc")     # (1-T)^2
            nc.scalar.activation(wc[:], omT[:], AF.Square)
            h = pC.tile([128, xb], f32, tag="h")       # E2 + 2T  (= 2*(E+T))
            nc.vector.scalar_tensor_tensor(h[:], T[:], 2.0, E2[:], OP.mult, OP.add)
            nc.gpsimd.tensor_tensor(h[:], rho[:], h[:], OP.mult)         # rhoH2
            qx = pC.tile([128, xb], f32, tag="qx")
            nc.gpsimd.tensor_tensor(qx[:], h[:], ux[:], OP.mult)
            qy = pC.tile([128, xb], f32, tag="qy")
            nc.gpsimd.tensor_tensor(qy[:], h[:], uy[:], OP.mult)
            # tau / omega / omegaT:  tau-1 = (K1/(rho T) + K0) * mask
            rhoT = pC.tile([128, xb], f32, tag="invr")
            nc.gpsimd.tensor_tensor(rhoT[:], rho[:], T[:], OP.mult)
            rr = pC.tile([128, xb], f32, tag="sqx")     # K1 / (rho*T)
            act_recip(rr[:], rhoT[:], scale=INV_K1)
            mask = pC.tile([128, xb], f32, tag="sqy")
            nc.vector.tensor_scalar(mask[:], acc[:], 9.0, None, OP.is_lt)
            tmw = pC.tile([128, xb], f32, tag="acc")   # tau - 1
            nc.vector.scalar_tensor_tensor(tmw[:], rr[:], K0, mask[:], OP.add, OP.mult)
            omg = pC.tile([128, xb], f32, tag="h")
            act_recip(omg[:], tmw[:], bias=1.0)                    # 1/tau
            omgT = pC.tile([128, xb], f32, tag="tmpA")
            act_recip(omgT[:], tmw[:], bias=C0T, scale=C1T)        # 1/tauT
            nc.scalar.mul(E2[:], E2[:], 0.5)           # E output

            # ---------------- F_post + streaming output ----------------
            for q in range(Qn):
                eng = nc.vector if q < 5 else nc.gpsimd
                eng.tensor_tensor(Dt[q][:], omg[:], Dt[q][:], OP.mult)
                eng.tensor_tensor(Dt[q][:], Ft[q][:], Dt[q][:], OP.subtract)

            # column segments for the periodic x shift
            def csegs(t):
                if t == 0:
                    return [(0, xb, x0)]
                if t == 1:
                    if x0 + xb == X:
                        return [(0, xb - 1, x0 + 1), (xb - 1, 1, 0)]
                    return [(0, xb, x0 + 1)]
                if x0 == 0:
                    return [(0, 1, X - 1), (1, xb - 1, 0)]
                return [(0, xb, x0 - 1)]

            for q in range(Qn):
                s = EY[q]
                if s == 1 and r0 == 0:
                    rsegs = [(0, 1, "x", EXTRA_TOP[q]), (1, 127, "m", 0)]
                elif s == -1 and r0 == 128:
                    rsegs = [(0, 127, "m", r0 + 1), (127, 1, "x", EXTRA_BOT[q])]
                else:
                    rsegs = [(0, 128, "m", r0 - s)]
                for (p0, np_, kind, dr) in rsegs:
                    for (c0, w, dc) in csegs(EX[q]):
                        src = Dt[q][p0:p0 + np_, c0:c0 + w]
                        if kind == "m":
                            nc.scalar.dma_start(out_ap[q, dr:dr + np_, dc:dc + w], src)
                        else:
                            nc.scalar.dma_start(ext_ap[dr, dc:dc + w], src)

            # w channels (broadcast 4/4/1) + field channels, on the ACT queue
            for ch in (9, 10, 11, 12):
                nc.scalar.dma_start(out_ap[ch, r0:r0 + 128, x0:x0 + xb], wa[:])
            for ch in (13, 14, 15, 16):
                nc.scalar.dma_start(out_ap[ch, r0:r0 + 128, x0:x0 + xb], sqT[:])
            nc.scalar.dma_start(out_ap[17, r0:r0 + 128, x0:x0 + xb], wc[:])
            for ch, t in ((18, rho), (19, ux), (20, uy), (21, E2), (22, T),
                          (23, qx), (24, qy), (25, omgT)):
                nc.scalar.dma_start(out_ap[ch, r0:r0 + 128, x0:x0 + xb], t[:])

        for r0 in (0, 128):
            x0 = 0
            for xb in XBLOCKS:
                block(r0, x0, xb)
                x0 += xb

    nc.compile()
    return nc


def _get_program():
    if "nc" not in _CACHE:
        _CACHE["nc"] = build_program()
    return _CACHE["nc"]


def kernel(F, G, Feq):
    from concourse.bass_utils import run_bass_kernel_spmd

    F = np.ascontiguousarray(np.asarray(F, np.float32))
    G = np.ascontiguousarray(np.asarray(G, np.float32))
    Feq = np.ascontiguousarray(np.asarray(Feq, np.float32))
    nc = _get_program()
    W = _esum_weights()
    in_maps = []
    for c in range(N_CORES):
        sl = slice(c * RPC, (c + 1) * RPC)
        in_maps.append({"F": F[:, sl, :], "G": G[:, sl, :], "Feq": Feq[:, sl, :],
                        "W": W})
    res = run_bass_kernel_spmd(nc, in_maps, core_ids=list(range(N_CORES)))
    out = np.empty((26, Y, X), np.float32)
    for c in range(N_CORES):
        out[:, c * RPC:(c + 1) * RPC, :] = res.results[c]["out"]
    for c in range(N_CORES):
        ex = res.results[c]["extra"]
        for q, i in EXTRA_TOP.items():
            out[q, (c * RPC - 1) % Y, :] = ex[i]
        for q, i in EXTRA_BOT.items():
            out[q, ((c + 1) * RPC) % Y, :] = ex[i]
    return out


# revision 10
# speedup vs baseline: 1.4078x; 1.3200x over previous
"""D2Q9 lattice-Boltzmann solver step (collision + moments + streaming) on 8
Trainium2 NeuronCores.

Sharding: the (Y, X) grid is split along Y into 8 contiguous slabs of 256
rows, one per core. All moment/collision math is local per cell; the
periodic-shift streaming step is realized purely through output DMA
addressing (write F_post row y to output row y-EY, column x+EX mod X). The
six F_str rows per core that fall outside the core's own output slab
(EY=+1 planes at the top edge, EY=-1 planes at the bottom edge) are written
to a small per-core `extra` tensor and placed by the host gather, so no
input halo or device-to-device communication is needed at all.

Per core the program runs 2 row-supertiles x 4 x-blocks of 512. Esum =
sum_q G runs on the TensorEngine per supertile (q-on-partition group
layout, 0/1 fp32 weights accumulated into PSUM). Per block: merged F/Feq
arena loads (one DMA each); d = F - Feq; r = |d| * recip(Feq + 1e-10) with
the bit-exact DVE reciprocal, accumulated in ascending q order
(threshold-critical: the measured margin min|EPS-1| is ~2e-7 relative);
rho/ux/uy shared-subexpression adds and F_post = F - omega*d on GpSimd;
smooth-field reciprocals (1/rho, tau path) on the ACT spline engine
(<=1.2e-5 rel err, none feed the EPS mask); w and moment fields are packed
into SBUF arenas so each group leaves in a single DMA.
"""
from contextlib import ExitStack

import numpy as np

# ---------------- problem constants (hardcoded per contract) ----------------
Qn, Y, X = 9, 2048, 2048
N_CORES = 8
RPC = Y // N_CORES  # 256 interior rows per core
XB = 512
EX = [1, 0, -1, 0, 1, -1, -1, 1, 0]
EY = [0, 1, 0, -1, 1, 1, -1, -1, 0]
# G-group layout for the Esum matmuls: (row offset, nrows); 9*14+9*2 = 128 rows
GROUPS = [(14 * g, 14) for g in range(9)] + [(126, 2)]
EXTRA_TOP = {1: 0, 4: 1, 5: 2}  # EY=+1: F_str global row y0-1  -> extra[idx]
EXTRA_BOT = {3: 3, 6: 4, 7: 5}  # EY=-1: F_str global row y0+256 -> extra[idx]

# ---- constants replicated in f32 exactly as the jax reference computes ----
_F = np.float32
ICV32 = float(_F(1.4 - 1.0))               # 0.40000000596... (f32 of 0.4-ish)
C_T = ICV32 / 2.0                          # T = C_T * (E2 - uu); 2*C_T == ICV32
K1 = float(_F(_F(1.35) * _F(0.01)))        # tau-1 = (K1/(rho T) + K0) * mask
K0 = float(_F(_F(1.35) * _F(0.5)) - _F(1.0))
INV_K1 = float(_F(1.0) / _F(K1))
C1T = float(_F(1.0) / _F(0.71))            # tauT = C1T * tmw + C0T
C0T = float(_F(0.5) + _F(_F(0.5) * _F(1.0) / _F(0.71)))
EPS_BIAS = float(_F(1e-10))

_CACHE = {}


def _esum_weights():
    """lhsT weights (10, 126, 128) f32: W[g][(q*rows+dy), 14*g+dy] = 1."""
    W = np.zeros((10, 126, 128), np.float32)
    for g, (r0, rows) in enumerate(GROUPS):
        for q in range(Qn):
            for dy in range(rows):
                W[g, q * rows + dy, r0 + dy] = 1.0
    return W


def build_program():
    import concourse.bass as bass  # noqa: F401
    import concourse.tile as tile
    from concourse import bacc, mybir

    f32 = mybir.dt.float32
    OP = mybir.AluOpType
    AF = mybir.ActivationFunctionType

    nc = bacc.Bacc("TRN2", target_bir_lowering=False, debug=False,
                   enable_asserts=False, num_devices=N_CORES)
    # extra const AP used as ACT bias (e = Feq + 1e-10)
    _ct = nc.alloc_sbuf_tensor("const-eps10", [128, 1], f32)
    nc.gpsimd.memset(_ct.ap(), EPS_BIAS)
    nc.const_aps.aps[(f32, EPS_BIAS)] = _ct.ap()
    nc.all_engine_barrier()

    F_ap = nc.dram_tensor("F", [Qn, RPC, X], f32, kind="ExternalInput").ap()
    G_ap = nc.dram_tensor("G", [Qn, RPC, X], f32, kind="ExternalInput").ap()
    Feq_ap = nc.dram_tensor("Feq", [Qn, RPC, X], f32, kind="ExternalInput").ap()
    W_ap = nc.dram_tensor("W", [10, 126, 128], f32, kind="ExternalInput").ap()
    out_ap = nc.dram_tensor("out", [26, RPC, X], f32, kind="ExternalOutput").ap()
    ext_ap = nc.dram_tensor("extra", [6, X], f32, kind="ExternalOutput").ap()

    def act_recip(out, in_, bias=0.0, scale=1.0):
        """Raw ACT-engine reciprocal: out = 1/(scale*in + bias).

        Spline-table implementation, measured <=1.2e-5 relative error —
        used only for smooth fields that never feed the EPS threshold.
        """
        nc.scalar.add_instruction(mybir.InstActivation(
            name=nc.get_next_instruction_name(),
            func=AF.Reciprocal,
            ins=[nc.scalar.lower_ap(in_),
                 mybir.ImmediateValue(dtype=f32, value=float(bias)),
                 mybir.ImmediateValue(dtype=f32, value=float(scale)),
                 mybir.ImmediateValue(dtype=f32, value=0.0)],
            outs=[nc.scalar.lower_ap(out)],
        ))

    with tile.TileContext(nc) as tc, ExitStack() as ctx:
        pW = ctx.enter_context(tc.tile_pool(name="w", bufs=1))
        pF = ctx.enter_context(tc.tile_pool(name="pf", bufs=2))    # F arena
        pQ = ctx.enter_context(tc.tile_pool(name="pq", bufs=2))    # Feq arena
        pD = ctx.enter_context(tc.tile_pool(name="pd", bufs=2))    # d tiles
        pL = ctx.enter_context(tc.tile_pool(name="pl", bufs=2))    # G group tiles
        pT = ctx.enter_context(tc.tile_pool(name="pt", bufs=2))    # e / ad rotating
        pC = ctx.enter_context(tc.tile_pool(name="pc", bufs=1))    # per-cell tags
        pA = ctx.enter_context(tc.tile_pool(name="pa", bufs=2))    # acc (block-pipelined)
        pP = ctx.enter_context(tc.tile_pool(name="pp", bufs=2, space="PSUM"))

        # stationary Esum weights, loaded once
        Wt = []
        for g, (_, rows) in enumerate(GROUPS):
            parts = Qn * rows
            wt = pW.tile([parts, 128], f32, tag=f"W{g}")
            nc.sync.dma_start(wt[:], W_ap[g, :parts, :])
            Wt.append(wt)

        def supertile(r0):
            # ---- Esum over q on the TensorEngine, whole 2048-wide stripe ----
            es = pP.tile([128, X], f32, tag="esum")
            for g, (gr0, rows) in enumerate(GROUPS):
                parts = Qn * rows
                gt = pL.tile([parts, X], f32, tag="g")
                nc.sync.dma_start(gt[:], G_ap[:, r0 + gr0:r0 + gr0 + rows, :])
                for n0 in range(0, X, 512):
                    nc.tensor.matmul(es[:, n0:n0 + 512], Wt[g][:parts, :],
                                     gt[:parts, n0:n0 + 512],
                                     start=(g == 0), stop=(g == 9))

            for x0 in range(0, X, XB):
                block(r0, x0, XB, es)

        def block(r0, x0, xb, es):
            # ---------------- merged loads ----------------
            farena = pF.tile([128, Qn * xb], f32, tag="farena")
            nc.sync.dma_start(
                farena[:].rearrange("p (q x) -> p q x", q=Qn),
                F_ap[:, r0:r0 + 128, x0:x0 + xb].rearrange("q r x -> r q x"))
            Ft = [farena[:, q * xb:(q + 1) * xb] for q in range(Qn)]

            qarena = pQ.tile([128, Qn * xb], f32, tag="qarena")
            nc.sync.dma_start(
                qarena[:].rearrange("p (q x) -> p q x", q=Qn),
                Feq_ap[:, r0:r0 + 128, x0:x0 + xb].rearrange("q r x -> r q x"))
            Feqt = [qarena[:, q * xb:(q + 1) * xb] for q in range(Qn)]

            # output arenas: w (9 channels) and moment fields (8 channels)
            war = pC.tile([128, 9 * xb], f32, tag="war")
            Wsl = [war[:, i * xb:(i + 1) * xb] for i in range(9)]
            fld = pC.tile([128, 8 * xb], f32, tag="fld")
            rho = fld[:, 0 * xb:1 * xb]
            ux = fld[:, 1 * xb:2 * xb]
            uy = fld[:, 2 * xb:3 * xb]
            E2 = fld[:, 3 * xb:4 * xb]
            T = fld[:, 4 * xb:5 * xb]
            qxs = fld[:, 5 * xb:6 * xb]
            qys = fld[:, 6 * xb:7 * xb]
            omgT = fld[:, 7 * xb:8 * xb]

            # -------- per-q: d, e=recip(Feq+1e-10), EPS acc (exact) ----------
            acc = pA.tile([128, xb], f32, tag="acc")
            Dt = []
            for q in range(Qn):
                d = pD.tile([128, xb], f32, tag=f"d{q}")
                nc.vector.tensor_tensor(d[:], Ft[q][:], Feqt[q][:], OP.subtract)
                Dt.append(d)
                e = pT.tile([128, xb], f32, tag="e")
                nc.scalar.activation(e[:], Feqt[q][:], AF.Identity, bias=EPS_BIAS)
                nc.vector.reciprocal(e[:], e[:])
                ad = pT.tile([128, xb], f32, tag="ad")
                nc.scalar.activation(ad[:], d[:], AF.Abs)
                if q == 0:
                    nc.vector.tensor_tensor(acc[:], ad[:], e[:], OP.mult)
                else:
                    nc.vector.tensor_tensor(ad[:], ad[:], e[:], OP.mult)
                    nc.vector.tensor_tensor(acc[:], acc[:], ad[:], OP.add)

            # ---------------- rho / ux / uy (GpSimd) ----------------
            sxp = pC.tile([128, xb], f32, tag="tmpA")   # F0+F4+F7
            nc.gpsimd.tensor_tensor(sxp[:], Ft[0][:], Ft[4][:], OP.add)
            nc.gpsimd.tensor_tensor(sxp[:], sxp[:], Ft[7][:], OP.add)
            sxm = pC.tile([128, xb], f32, tag="tmpB")   # F2+F5+F6
            nc.gpsimd.tensor_tensor(sxm[:], Ft[2][:], Ft[5][:], OP.add)
            nc.gpsimd.tensor_tensor(sxm[:], sxm[:], Ft[6][:], OP.add)
            s138 = pC.tile([128, xb], f32, tag="tmpC")  # F1+F3+F8
            nc.gpsimd.tensor_tensor(s138[:], Ft[1][:], Ft[3][:], OP.add)
            nc.gpsimd.tensor_tensor(s138[:], s138[:], Ft[8][:], OP.add)
            nc.gpsimd.tensor_tensor(rho[:], sxp[:], sxm[:], OP.add)
            nc.gpsimd.tensor_tensor(rho[:], rho[:], s138[:], OP.add)
            uxn = pC.tile([128, xb], f32, tag="uxn")
            nc.gpsimd.tensor_tensor(uxn[:], sxp[:], sxm[:], OP.subtract)
            syp = pC.tile([128, xb], f32, tag="tmpC")   # F1+F4+F5
            nc.gpsimd.tensor_tensor(syp[:], Ft[4][:], Ft[5][:], OP.add)
            nc.gpsimd.tensor_tensor(syp[:], syp[:], Ft[1][:], OP.add)
            sym = pC.tile([128, xb], f32, tag="tmpB")   # F3+F6+F7
            nc.gpsimd.tensor_tensor(sym[:], Ft[6][:], Ft[7][:], OP.add)
            nc.gpsimd.tensor_tensor(sym[:], sym[:], Ft[3][:], OP.add)
            uyn = pC.tile([128, xb], f32, tag="uyn")
            nc.gpsimd.tensor_tensor(uyn[:], syp[:], sym[:], OP.subtract)

            # ---------------- per-cell fields ----------------
            invr = pC.tile([128, xb], f32, tag="invr")
            act_recip(invr[:], rho[:])                 # ~1e-5, smooth-only
            nc.gpsimd.tensor_tensor(ux[:], uxn[:], invr[:], OP.mult)
            nc.gpsimd.tensor_tensor(uy[:], uyn[:], invr[:], OP.mult)
            nc.vector.tensor_tensor(E2[:], es[:, x0:x0 + xb], invr[:], OP.mult)
            sqx = pC.tile([128, xb], f32, tag="sqx")
            nc.scalar.activation(sqx[:], ux[:], AF.Square)
            sqy = pC.tile([128, xb], f32, tag="sqy")
            nc.scalar.activation(sqy[:], uy[:], AF.Square)
            nc.gpsimd.tensor_tensor(sqx[:], sqx[:], sqy[:], OP.add)      # uu
            nc.vector.tensor_tensor(sqx[:], E2[:], sqx[:], OP.subtract)  # E2-uu
            nc.vector.tensor_scalar(T[:], sqx[:], C_T, 1e-6, OP.mult, OP.max)
            omT = pC.tile([128, xb], f32, tag="omT")   # 1 - T
            nc.scalar.activation(omT[:], T[:], AF.Copy, bias=1.0, scale=-1.0)
            # w: wa = 0.5*T*(1-T) (x4), wb = (0.5*T)^2 (x4), wc = (1-T)^2
            nc.vector.scalar_tensor_tensor(Wsl[0][:], T[:], 0.5, omT[:],
                                           OP.mult, OP.mult)
            nc.scalar.activation(Wsl[4][:], T[:], AF.Square, scale=0.5)
            nc.scalar.activation(Wsl[8][:], omT[:], AF.Square)
            for i in (1, 2, 3):
                nc.scalar.copy(Wsl[i][:], Wsl[0][:])
            for i in (5, 6, 7):
                nc.scalar.copy(Wsl[i][:], Wsl[4][:])
            h = pC.tile([128, xb], f32, tag="h")       # E2 + 2T  (= 2*(E+T))
            nc.vector.scalar_tensor_tensor(h[:], T[:], 2.0, E2[:], OP.mult, OP.add)
            nc.gpsimd.tensor_tensor(h[:], rho[:], h[:], OP.mult)         # rhoH2
            nc.gpsimd.tensor_tensor(qxs[:], h[:], ux[:], OP.mult)
            nc.gpsimd.tensor_tensor(qys[:], h[:], uy[:], OP.mult)
            # tau / omega / omegaT:  tau-1 = (K1/(rho T) + K0) * mask
            rhoT = pC.tile([128, xb], f32, tag="invr")
            nc.gpsimd.tensor_tensor(rhoT[:], rho[:], T[:], OP.mult)
            rr = pC.tile([128, xb], f32, tag="sqx")    # K1 / (rho*T)
            act_recip(rr[:], rhoT[:], scale=INV_K1)
            mask = pC.tile([128, xb], f32, tag="sqy")
            nc.vector.tensor_scalar(mask[:], acc[:], 9.0, None, OP.is_lt)
            tmw = pC.tile([128, xb], f32, tag="tmw")   # tau - 1
            nc.vector.scalar_tensor_tensor(tmw[:], rr[:], K0, mask[:], OP.add, OP.mult)
            omg = pC.tile([128, xb], f32, tag="h")
            act_recip(omg[:], tmw[:], bias=1.0)                    # 1/tau
            act_recip(omgT[:], tmw[:], bias=C0T, scale=C1T)        # 1/tauT
            nc.scalar.mul(E2[:], E2[:], 0.5)           # E output

            # ---------------- F_post + streaming output ----------------
            for q in range(Qn):
                nc.gpsimd.tensor_tensor(Dt[q][:], omg[:], Dt[q][:], OP.mult)
                nc.gpsimd.tensor_tensor(Dt[q][:], Ft[q][:], Dt[q][:], OP.subtract)

            # column segments for the periodic x shift
            def csegs(t):
                if t == 0:
                    return [(0, xb, x0)]
                if t == 1:
                    if x0 + xb == X:
                        return [(0, xb - 1, x0 + 1), (xb - 1, 1, 0)]
                    return [(0, xb, x0 + 1)]
                if x0 == 0:
                    return [(0, 1, X - 1), (1, xb - 1, 0)]
                return [(0, xb, x0 - 1)]

            for q in range(Qn):
                s = EY[q]
                if s == 1 and r0 == 0:
                    rsegs = [(0, 1, "x", EXTRA_TOP[q]), (1, 127, "m", 0)]
                elif s == -1 and r0 == 128:
                    rsegs = [(0, 127, "m", r0 + 1), (127, 1, "x", EXTRA_BOT[q])]
                else:
                    rsegs = [(0, 128, "m", r0 - s)]
                for (p0, np_, kind, dr) in rsegs:
                    for (c0, w, dc) in csegs(EX[q]):
                        src = Dt[q][p0:p0 + np_, c0:c0 + w]
                        if kind == "m":
                            nc.sync.dma_start(out_ap[q, dr:dr + np_, dc:dc + w], src)
                        else:
                            nc.sync.dma_start(ext_ap[dr, dc:dc + w], src)

            # single-DMA w and field channel groups
            nc.scalar.dma_start(
                out_ap[9:18, r0:r0 + 128, x0:x0 + xb].rearrange("c r x -> r c x"),
                war[:].rearrange("p (c x) -> p c x", c=9))
            nc.scalar.dma_start(
                out_ap[18:26, r0:r0 + 128, x0:x0 + xb].rearrange("c r x -> r c x"),
                fld[:].rearrange("p (c x) -> p c x", c=8))

        for r0 in (0, 128):
            supertile(r0)

    nc.compile()
    return nc


def _get_program():
    if "nc" not in _CACHE:
        _CACHE["nc"] = build_program()
    return _CACHE["nc"]


def kernel(F, G, Feq):
    from concourse.bass_utils import run_bass_kernel_spmd

    F = np.ascontiguousarray(np.asarray(F, np.float32))
    G = np.ascontiguousarray(np.asarray(G, np.float32))
    Feq = np.ascontiguousarray(np.asarray(Feq, np.float32))
    nc = _get_program()
    W = _esum_weights()
    in_maps = []
    for c in range(N_CORES):
        sl = slice(c * RPC, (c + 1) * RPC)
        in_maps.append({"F": F[:, sl, :], "G": G[:, sl, :], "Feq": Feq[:, sl, :],
                        "W": W})
    res = run_bass_kernel_spmd(nc, in_maps, core_ids=list(range(N_CORES)))
    out = np.empty((26, Y, X), np.float32)
    for c in range(N_CORES):
        out[:, c * RPC:(c + 1) * RPC, :] = res.results[c]["out"]
    for c in range(N_CORES):
        ex = res.results[c]["extra"]
        for q, i in EXTRA_TOP.items():
            out[q, (c * RPC - 1) % Y, :] = ex[i]
        for q, i in EXTRA_BOT.items():
            out[q, ((c + 1) * RPC) % Y, :] = ex[i]
    return out
